# revision 9
# baseline (speedup 1.0000x reference)
"""Trainium2 Bass kernel for nn_MixedRationalQuadraticCouplingTransform.

Contract: kernel(**inputs) takes FULL inputs (N=65536) and returns the full
(outputs [N,128] f32, logabsdet [N] f32) tuple, computed on 8 NeuronCores in
pure data parallel (batch sharded 8 ways, weights replicated).

Per-core program (Tile framework, built once and run SPMD on cores 0-7):
  - 16 tiles of 512 samples.
  - net_in = [cos(ic), sin(ic), ir, ctx] built sample-major (fp16), transposed
    to feature-major via PE transposes.
  - 3-layer MLP in feature-major with fp16 operands, fp32 PSUM accumulation.
    Layer 3 uses the (feature-major) h2 as the stationary operand so params
    come out sample-major, directly consumable by the spline stage. W3 columns
    are host-permuted into [uw_c|uh_c|ud_c|pad|uw_r|uh_r|ud_r] blocks and b3
    is applied via an appended all-ones contraction row.
  - Rational-quadratic spline evaluated with mask-dot gathers (monotone bin
    masks m_j = [x >= knot_j]; gathered q = q_0 + sum_j m_j dq_j) and a
    division-free rational form (single reciprocal per output).
"""
import numpy as np

import concourse.bacc as bacc
import concourse.tile as tile
from concourse import mybir
from concourse.bass_utils import run_bass_kernel_spmd

F32 = mybir.dt.float32
F16 = mybir.dt.float16
AX = mybir.AxisListType
OP = mybir.AluOpType
AF = mybir.ActivationFunctionType

N_FULL = 65536
N_CORES = 8
N_CORE = N_FULL // N_CORES          # 8192
S_TILE = 512                        # samples per tile
C_SUB = S_TILE // 128               # 4 sub-chunks of 128 samples
PI = float(np.pi)
SCALE = float(1.0 / np.sqrt(512.0))
MIN_BW = 1e-3
MIN_D = 1e-3
K = 5
TAIL = 5.0


def _emit_spline(nc, pools, pfx, x_sl, ew_t, eh_t, sp_t, nd,
                 left, right, out_view, lad_view, xl_bias):
    """Emit spline evaluation ops for one spline type over a [128, C, 32] slab.

    x_sl: [128, C, 32] f32 AP of the transform inputs.
    ew_t/eh_t: [128, C, 160] f32 tiles holding exp(scale*uw), exp(scale*uh).
    sp_t: [128, C, 32*nd] f32 tile holding softplus(ud) (nd=5 circular, 6 regular).
    out_view / lad_view: [128, C, 32] f32 APs to write y and logabsdet parts.
    """
    vec, gps, act = nc.vector, nc.gpsimd, nc.scalar
    sm, lg = pools["sm"], pools["lg"]
    C = C_SUB
    aW = (right - left) * (1.0 - MIN_BW * K)
    c1 = (right - left) * MIN_BW
    bottom = left

    ew4 = ew_t[:].rearrange("p c (d j) -> p c d j", j=5)
    eh4 = eh_t[:].rearrange("p c (d j) -> p c d j", j=5)
    sp4 = sp_t[:].rearrange("p c (d j) -> p c d j", j=nd)

    def smt(name):
        return sm.tile([128, C, 32], F32, tag=f"{pfx}{name}", name=f"{pfx}{name}")

    def lgt(name, j):
        return lg.tile([128, C, 32, j], F32, tag=f"{pfx}{name}", name=f"{pfx}{name}")

    # widths / heights: w_j = c1 + aW * e_j / E
    Ew = smt("Ew")
    vec.tensor_reduce(Ew[:], ew4, axis=AX.X, op=OP.add)
    rEw = smt("rEw")
    vec.reciprocal_approx_fast(rEw[:], Ew[:])
    w_t = lgt("w", 5)
    vec.scalar_tensor_tensor(w_t[:], ew4, aW,
                             rEw[:].unsqueeze(3).broadcast_to([128, C, 32, 5]),
                             op0=OP.mult, op1=OP.mult)
    vec.tensor_scalar_add(w_t[:], w_t[:], c1)

    Eh = smt("Eh")
    vec.tensor_reduce(Eh[:], eh4, axis=AX.X, op=OP.add)
    rEh = smt("rEh")
    vec.reciprocal_approx_fast(rEh[:], Eh[:])
    h_t = lgt("h", 5)
    vec.scalar_tensor_tensor(h_t[:], eh4, aW,
                             rEh[:].unsqueeze(3).broadcast_to([128, C, 32, 5]),
                             op0=OP.mult, op1=OP.mult)
    vec.tensor_scalar_add(h_t[:], h_t[:], c1)

    # knots (cumsum of widths) and bin masks
    Ct = lgt("C", 4)
    gps.tensor_copy(Ct[:, :, :, 0], w_t[:, :, :, 0])
    for j in range(1, 4):
        gps.tensor_add(Ct[:, :, :, j], Ct[:, :, :, j - 1], w_t[:, :, :, j])
    xl = smt("xl")
    act.activation(xl[:], x_sl, AF.Identity, bias=xl_bias[:])
    m_t = lgt("m", 4)
    vec.tensor_tensor(m_t[:], xl[:].unsqueeze(3).broadcast_to([128, C, 32, 4]),
                      Ct[:], op=OP.is_ge)

    # derivative prep: sp += MIN_D, dd_j = d_{j+1} - d_j (with circular wrap)
    vec.tensor_scalar_add(sp_t[:], sp_t[:], MIN_D)
    dd = lgt("dd", 5)
    if nd == 5:
        gps.tensor_tensor(dd[:, :, :, 0:4], sp4[:, :, :, 1:5], sp4[:, :, :, 0:4],
                          op=OP.subtract)
        gps.tensor_tensor(dd[:, :, :, 4], sp4[:, :, :, 0], sp4[:, :, :, 4],
                          op=OP.subtract)
    else:
        gps.tensor_tensor(dd[:], sp4[:, :, :, 1:6], sp4[:, :, :, 0:5],
                          op=OP.subtract)

    # mask-dot gathers
    def gather(name, val4, base, diff=None, eng=vec):
        """out = base + sum_j m_j * val4_j   ([128,C,32,4] views)."""
        tmp = lg.tile([128, C, 32, 4], F32, tag="gtmp", name="gtmp", bufs=2)
        eng.tensor_tensor(tmp[:], m_t[:], val4, op=OP.mult)
        red = smt("red" + name)
        vec.tensor_reduce(red[:], tmp[:], axis=AX.X, op=OP.add)
        if base is None:
            return red
        out = smt(name)
        vec.tensor_add(out[:], base, red[:])
        return out

    g_cw = gather("gcw", w_t[:, :, :, 0:4], None)
    s = smt("s")
    vec.tensor_tensor(s[:], xl[:], g_cw[:], op=OP.subtract)

    dw = lgt("dw", 4)
    gps.tensor_tensor(dw[:], w_t[:, :, :, 1:5], w_t[:, :, :, 0:4], op=OP.subtract)
    in_w = gather("inw", dw[:], w_t[:, :, :, 0])

    dh = lgt("dh", 4)
    gps.tensor_tensor(dh[:], h_t[:, :, :, 1:5], h_t[:, :, :, 0:4], op=OP.subtract)
    in_h = gather("inh", dh[:], h_t[:, :, :, 0])

    ch0 = gather("ch0", h_t[:, :, :, 0:4], None)
    d_k = gather("dk", dd[:, :, :, 0:4], sp4[:, :, :, 0])
    d_k1 = gather("dk1", dd[:, :, :, 1:5], sp4[:, :, :, 1])

    # rational part (division-free: everything scaled by in_w^3)
    def bin2(name, a, b, op, eng=vec):
        t = smt(name)
        eng.tensor_tensor(t[:], a, b, op=op)
        return t

    u = bin2("u", in_w[:], s[:], OP.subtract)
    su = bin2("su", s[:], u[:], OP.mult)
    s2 = bin2("s2", s[:], s[:], OP.mult, gps)
    w2 = bin2("w2", in_w[:], in_w[:], OP.mult, gps)
    u2 = bin2("u2", u[:], u[:], OP.mult, gps)
    dkw = bin2("dkw", d_k[:], in_w[:], OP.mult, gps)
    dk1w = bin2("dk1w", d_k1[:], in_w[:], OP.mult, gps)
    tsum = bin2("tsum", dkw[:], dk1w[:], OP.add, gps)
    hw2 = bin2("hw2", in_h[:], w2[:], OP.mult)
    t2 = smt("t2")
    vec.scalar_tensor_tensor(t2[:], in_h[:], -2.0, tsum[:], op0=OP.mult, op1=OP.add)
    t2su = bin2("t2su", t2[:], su[:], OP.mult)
    D3 = bin2("D3", hw2[:], t2su[:], OP.add)
    n1 = bin2("n1", in_h[:], s2[:], OP.mult)
    n2 = bin2("n2", dkw[:], su[:], OP.mult)
    n12 = bin2("n12", n1[:], n2[:], OP.add)
    num = bin2("num", in_h[:], n12[:], OP.mult)
    rD3 = smt("rD3")
    vec.reciprocal_approx_fast(rD3[:], D3[:])
    y0 = bin2("y0", num[:], rD3[:], OP.mult)
    y1 = bin2("y1", y0[:], ch0[:], OP.add)
    vec.tensor_scalar_add(out_view, y1[:], bottom)

    # logabsdet = 2 ln(in_h) + ln(in_w) + ln(Q) - 2 ln(D3)
    q1 = bin2("q1", dk1w[:], s2[:], OP.mult)
    q2 = smt("q2")
    vec.scalar_tensor_tensor(q2[:], in_h[:], 2.0, su[:], op0=OP.mult, op1=OP.mult)
    q3 = bin2("q3", dkw[:], u2[:], OP.mult, gps)
    q4 = bin2("q4", q1[:], q2[:], OP.add)
    Q = bin2("Q", q4[:], q3[:], OP.add)
    lnh = smt("lnh")
    act.activation(lnh[:], in_h[:], AF.Ln)
    lnw = smt("lnw")
    act.activation(lnw[:], in_w[:], AF.Ln)
    lnQ = smt("lnQ")
    act.activation(lnQ[:], Q[:], AF.Ln)
    lnD = smt("lnD")
    act.activation(lnD[:], D3[:], AF.Ln)
    l1 = smt("l1")
    vec.scalar_tensor_tensor(l1[:], lnh[:], 2.0, lnw[:], op0=OP.mult, op1=OP.add)
    l2 = smt("l2")
    vec.scalar_tensor_tensor(l2[:], lnD[:], -2.0, lnQ[:], op0=OP.mult, op1=OP.add)
    vec.tensor_add(lad_view, l1[:], l2[:])


def build_nc(n_core=N_CORE):
    """Build the per-core Tile program. Same program runs on all 8 cores."""
    n_tiles = n_core // S_TILE
    C = C_SUB
    nc = bacc.Bacc("TRN2", target_bir_lowering=False, debug=False,
                   enable_asserts=True, num_devices=1)

    d_in = nc.dram_tensor("inputs", [n_core, 128], F32, kind="ExternalInput").ap()
    d_net = nc.dram_tensor("net16", [n_core, 160], F16, kind="ExternalInput").ap()
    d_w1 = nc.dram_tensor("W1h", [160, 512], F16, kind="ExternalInput").ap()
    d_w2 = nc.dram_tensor("W2h", [512, 512], F16, kind="ExternalInput").ap()
    d_w3 = nc.dram_tensor("W3b", [513, 1024], F16, kind="ExternalInput").ap()
    d_b1 = nc.dram_tensor("b1r", [128, 4], F32, kind="ExternalInput").ap()
    d_b2 = nc.dram_tensor("b2r", [128, 4], F32, kind="ExternalInput").ap()
    d_id = nc.dram_tensor("ident", [128, 128], F16, kind="ExternalInput").ap()
    d_out = nc.dram_tensor("outputs", [n_core, 128], F32, kind="ExternalOutput").ap()
    d_lad = nc.dram_tensor("lad", [n_core], F32, kind="ExternalOutput").ap()

    with tile.TileContext(nc) as tc:
        with tc.tile_pool(name="const", bufs=1) as cst, \
             tc.tile_pool(name="mlp", bufs=2) as mlp, \
             tc.tile_pool(name="sm", bufs=1) as sm, \
             tc.tile_pool(name="lg", bufs=1) as lg, \
             tc.tile_pool(name="pt", bufs=2, space="PSUM") as ppt, \
             tc.tile_pool(name="pmlp", bufs=3, space="PSUM") as pmlp, \
             tc.tile_pool(name="pl3", bufs=3, space="PSUM") as pl3:
            pools = {"sm": sm, "lg": lg}

            # --- constants ---
            w1t = cst.tile([128, 512], F16)
            nc.sync.dma_start(w1t[:], d_w1[0:128, :])
            w1b = cst.tile([32, 512], F16)
            nc.sync.dma_start(w1b[:], d_w1[128:160, :])
            w2t = cst.tile([128, 4, 512], F16)
            nc.sync.dma_start(w2t[:], d_w2.rearrange("(k p) h -> p k h", p=128))
            w3t = cst.tile([128, 4, 1024], F16)
            nc.sync.dma_start(w3t[:], d_w3[0:512, :].rearrange("(k p) n -> p k n", p=128))
            w3l = cst.tile([1, 1024], F16)
            nc.sync.dma_start(w3l[:], d_w3[512:513, :])
            b1t = cst.tile([128, 4], F32)
            nc.sync.dma_start(b1t[:], d_b1[:])
            b2t = cst.tile([128, 4], F32)
            nc.sync.dma_start(b2t[:], d_b2[:])
            idt = cst.tile([128, 128], F16)
            nc.sync.dma_start(idt[:], d_id[:])
            ones = cst.tile([1, 128], F16)
            nc.gpsimd.memset(ones[:], 1.0)
            bias_pi = cst.tile([128, 1], F32)
            nc.gpsimd.memset(bias_pi[:], PI)
            bias_tail = cst.tile([128, 1], F32)
            nc.gpsimd.memset(bias_tail[:], TAIL)

            for t in range(n_tiles):
                r0 = t * S_TILE
                # --- input ---
                x_t = sm.tile([128, C, 128], F32, tag="x", bufs=2)
                nc.sync.dma_start(
                    x_t[:], d_in[r0:r0 + S_TILE, :].rearrange("(c p) f -> p c f", p=128))
                nin = sm.tile([128, C, 160], F16, tag="nin", bufs=2)
                nc.sync.dma_start(
                    nin[:],
                    d_net[r0:r0 + S_TILE, :].rearrange("(c p) f -> p c f", p=128))

                # --- transpose net_in to feature-major ---
                ntA = sm.tile([128, C, 128], F16, tag="ntA", bufs=2)
                ntB = sm.tile([32, C, 128], F16, tag="ntB", bufs=2)
                for c in range(C):
                    pt = ppt.tile([128, 256], F16, tag="pt")
                    nc.tensor.transpose(pt[:, 0:128], nin[:, c, 0:128], idt[:])
                    nc.tensor.transpose(pt[0:32, 128:256], nin[:, c, 128:160], idt[:])
                    nc.scalar.copy(ntA[:, c, :], pt[:, 0:128])
                    nc.scalar.copy(ntB[:, c, :], pt[0:32, 128:256])
                ntA_f = ntA[:].rearrange("p c s -> p (c s)")
                ntB_f = ntB[:].rearrange("p c s -> p (c s)")

                # --- layer 1 ---
                h1t = mlp.tile([128, 4, 512], F16, tag="h1")
                for mi in range(4):
                    p1 = pmlp.tile([128, 512], F32, tag="pmlp")
                    nc.tensor.matmul(p1[:], w1t[:, mi * 128:(mi + 1) * 128], ntA_f,
                                     start=True, stop=False)
                    nc.tensor.matmul(p1[:], w1b[:, mi * 128:(mi + 1) * 128], ntB_f,
                                     start=False, stop=True)
                    nc.scalar.activation(h1t[:, mi, :], p1[:], AF.Relu,
                                         bias=b1t[:, mi:mi + 1])

                # --- layer 2 ---
                h2t = mlp.tile([128, 4, 512], F16, tag="h2")
                for mi in range(4):
                    p2 = pmlp.tile([128, 512], F32, tag="pmlp")
                    for k in range(4):
                        nc.tensor.matmul(p2[:], w2t[:, k, mi * 128:(mi + 1) * 128],
                                         h1t[:, k, :], start=(k == 0), stop=(k == 3))
                    nc.scalar.activation(h2t[:, mi, :], p2[:], AF.Relu,
                                         bias=b2t[:, mi:mi + 1])

                # --- layer 3 (sample-major out) + spline param extraction ---
                ew_c = lg.tile([128, C, 160], F32, tag="ew_c", bufs=2)
                eh_c = lg.tile([128, C, 160], F32, tag="eh_c", bufs=2)
                sp_c = lg.tile([128, C, 160], F32, tag="sp_c", bufs=2)
                ew_r = lg.tile([128, C, 160], F32, tag="ew_r", bufs=2)
                eh_r = lg.tile([128, C, 160], F32, tag="eh_r", bufs=2)
                sp_r = lg.tile([128, C, 192], F32, tag="sp_r", bufs=2)
                for c in range(C):
                    p3a = pl3.tile([128, 512], F32, tag="pl3")
                    p3b = pl3.tile([128, 512], F32, tag="pl3")
                    for k in range(4):
                        lhsT = h2t[:, k, c * 128:(c + 1) * 128]
                        nc.tensor.matmul(p3a[:], lhsT, w3t[:, k, 0:512],
                                         start=(k == 0), stop=False)
                        nc.tensor.matmul(p3b[:], lhsT, w3t[:, k, 512:1024],
                                         start=(k == 0), stop=False)
                    nc.tensor.matmul(p3a[:], ones[:], w3l[:, 0:512],
                                     start=False, stop=True)
                    nc.tensor.matmul(p3b[:], ones[:], w3l[:, 512:1024],
                                     start=False, stop=True)
                    # nonlinearities straight out of PSUM
                    nc.scalar.activation(ew_c[:, c, :], p3a[:, 0:160], AF.Exp,
                                         scale=SCALE)
                    nc.scalar.activation(eh_c[:, c, :], p3a[:, 160:320], AF.Exp,
                                         scale=SCALE)
                    nc.scalar.activation(sp_c[:, c, :], p3a[:, 320:480], AF.Exp)
                    nc.scalar.activation(sp_c[:, c, :], sp_c[:, c, :], AF.Ln,
                                         bias=1.0)
                    nc.scalar.activation(ew_r[:, c, :], p3b[:, 0:160], AF.Exp,
                                         scale=SCALE)
                    nc.scalar.activation(eh_r[:, c, :], p3b[:, 160:320], AF.Exp,
                                         scale=SCALE)
                    nc.scalar.activation(sp_r[:, c, :], p3b[:, 320:512], AF.Exp)
                    nc.scalar.activation(sp_r[:, c, :], sp_r[:, c, :], AF.Ln,
                                         bias=1.0)

                # --- splines + output assembly ---
                out_t = sm.tile([128, C, 128], F32, tag="out", bufs=2)
                lad_t = sm.tile([128, C, 64], F32, tag="lad", bufs=2)
                nc.scalar.copy(out_t[:, :, 0:32], x_t[:, :, 0:32])
                nc.scalar.copy(out_t[:, :, 64:96], x_t[:, :, 64:96])
                _emit_spline(nc, pools, "c", x_t[:, :, 32:64], ew_c, eh_c, sp_c, 5,
                             -PI, PI, out_t[:, :, 32:64], lad_t[:, :, 0:32],
                             bias_pi)
                _emit_spline(nc, pools, "r", x_t[:, :, 96:128], ew_r, eh_r, sp_r, 6,
                             -TAIL, TAIL, out_t[:, :, 96:128], lad_t[:, :, 32:64],
                             bias_tail)

                nc.sync.dma_start(
                    d_out[r0:r0 + S_TILE, :].rearrange("(c p) f -> p c f", p=128),
                    out_t[:])
                lsum = sm.tile([128, C], F32, tag="lsum", bufs=2)
                nc.vector.tensor_reduce(lsum[:], lad_t[:], axis=AX.X, op=OP.add)
                nc.sync.dma_start(
                    d_lad[r0:r0 + S_TILE].rearrange("(c p) -> p c", p=128), lsum[:])

    nc.compile()
    return nc


def prep_weights(W1, b1, W2, b2, W3, b3):
    """Host-side: cast to fp16, permute+pad W3 columns, append b3 row."""
    # new column order: [uw_c(160) uh_c(160) ud_c(160) pad(32) uw_r(160) uh_r(160) ud_r(192)]
    perm = np.zeros(1024, dtype=np.int64)
    valid = np.zeros(1024, dtype=bool)
    for d in range(32):
        for j in range(5):
            perm[0 + d * 5 + j] = d * 15 + j          # uw_c
            perm[160 + d * 5 + j] = d * 15 + 5 + j    # uh_c
            perm[320 + d * 5 + j] = d * 15 + 10 + j   # ud_c
            perm[512 + d * 5 + j] = 480 + d * 16 + j        # uw_r
            perm[672 + d * 5 + j] = 480 + d * 16 + 5 + j    # uh_r
        for j in range(6):
            perm[832 + d * 6 + j] = 480 + d * 16 + 10 + j   # ud_r
    valid[0:480] = True
    valid[512:1024] = True

    W3p = np.zeros((512, 1024), np.float32)
    b3p = np.zeros((1024,), np.float32)
    W3p[:, valid] = W3[:, perm[valid]]
    b3p[valid] = b3[perm[valid]]
    W3b = np.concatenate([W3p, b3p[None, :]], axis=0).astype(np.float16)

    return {
        "W1h": W1.astype(np.float16),
        "W2h": W2.astype(np.float16),
        "W3b": W3b,
        "b1r": b1.reshape(4, 128).T.copy().astype(np.float32),
        "b2r": b2.reshape(4, 128).T.copy().astype(np.float32),
        "ident": np.eye(128, dtype=np.float16),
    }


_NC_CACHE = {}


def _get_nc(n_core):
    if n_core not in _NC_CACHE:
        _NC_CACHE[n_core] = build_nc(n_core)
    return _NC_CACHE[n_core]


def kernel(inputs, context, W1, b1, W2, b2, W3, b3):
    inputs = np.ascontiguousarray(np.asarray(inputs, dtype=np.float32))
    context = np.asarray(context, dtype=np.float32)
    wmap = prep_weights(np.asarray(W1, np.float32), np.asarray(b1, np.float32),
                        np.asarray(W2, np.float32), np.asarray(b2, np.float32),
                        np.asarray(W3, np.float32), np.asarray(b3, np.float32))
    ic = inputs[:, 0:32]
    net16 = np.concatenate(
        [np.cos(ic), np.sin(ic), inputs[:, 64:96], context],
        axis=1).astype(np.float16)

    nc = _get_nc(N_CORE)
    in_maps = []
    for c in range(N_CORES):
        sl = slice(c * N_CORE, (c + 1) * N_CORE)
        m = dict(wmap)
        m["inputs"] = inputs[sl]
        m["net16"] = net16[sl]
        in_maps.append(m)

    res = run_bass_kernel_spmd(nc, in_maps, list(range(N_CORES)))
    outputs = np.concatenate([res.results[c]["outputs"] for c in range(N_CORES)], 0)
    lad = np.concatenate([res.results[c]["lad"] for c in range(N_CORES)], 0)
    return outputs, lad


# revision 19
# speedup vs baseline: 1.2074x; 1.2074x over previous
"""Trainium2 Bass kernel for nn_MixedRationalQuadraticCouplingTransform.

kernel(**inputs) takes FULL inputs (N=65536), returns (outputs [N,128] f32,
logabsdet [N] f32). Pure data parallel on 8 NeuronCores: batch sharded 8 ways,
weights replicated.

Per-core program (Tile framework):
  - 16 tiles of 512 samples (4 sub-chunks of 128).
  - net_in (cos/sin/ir/ctx) is precomputed on host; on-chip it is transposed
    to feature-major via PE transposes, then a 3-layer MLP runs with fp32r
    matmul operands (full-rate on the PE, ~1.6e-4 matmul precision).
    Layer 3 uses feature-major h2 as the stationary operand so the 992+pad
    spline params land sample-major in PSUM, with W3 columns host-permuted
    into [uw_c|uh_c|ud_c|pad|uw_r|uh_r|ud_r] blocks and b3 applied via an
    appended ones-row contraction.
  - Both splines (circular + regular) are evaluated by shared ops over a
    64-wide feature axis with per-feature constant tiles; bin search is
    mask-based (m_j = [x >= knot_j]) and gathers are mask-dot products
    evaluated with a pairwise adder tree. All activation functions used
    (Exp/Ln/Relu/Copy/Identity/Square) live in one ACT table so the table is
    loaded exactly once; softplus(x) = Ln(Exp(x) + 1).
"""
import numpy as np

import concourse.bacc as bacc
import concourse.tile as tile
from concourse import mybir
from concourse.bass_utils import run_bass_kernel_spmd

F32 = mybir.dt.float32
F16 = mybir.dt.float16
F32R = mybir.dt.float32r
AX = mybir.AxisListType
OP = mybir.AluOpType
AF = mybir.ActivationFunctionType

N_FULL = 65536
N_CORES = 8
N_CORE = N_FULL // N_CORES          # 8192
S_TILE = 512
C_SUB = S_TILE // 128               # 4
PI = float(np.pi)
SCALE = float(1.0 / np.sqrt(512.0))
MIN_BW = 1e-3
MIN_D = 1e-3
K = 5
TAIL = 5.0
DT_W = F32   # dtype of wide spline tensors

_TABLE_PATCHED = False


def _patch_single_act_table():
    """Force the act-table pass to use only natural_log_exp_and_others
    (covers Exp/Ln/Relu/Copy/Identity/Square) so the table loads once."""
    global _TABLE_PATCHED
    if _TABLE_PATCHED:
        return
    from concourse.hw_specs import get_activation_tables as _orig

    def single(arch):
        tabs = _orig(arch)
        keep = "natural_log_exp_and_others"
        return {k: (v if k == keep else set()) for k, v in tabs.items()}

    bacc.get_activation_tables = single
    _TABLE_PATCHED = True


def build_nc(n_core=N_CORE):
    _patch_single_act_table()
    n_tiles = n_core // S_TILE
    C = C_SUB
    CD = C * 64
    nc = bacc.Bacc("TRN2", target_bir_lowering=False, debug=False,
                   enable_asserts=True, num_devices=1)

    d_in = nc.dram_tensor("inputs", [n_core, 128], F32, kind="ExternalInput").ap()
    d_net = nc.dram_tensor("net32", [n_core, 160], F32R, kind="ExternalInput").ap()
    d_w1 = nc.dram_tensor("W1f", [160, 512], F32R, kind="ExternalInput").ap()
    d_w2 = nc.dram_tensor("W2f", [512, 512], F32R, kind="ExternalInput").ap()
    d_w3 = nc.dram_tensor("W3b", [513, 1024], F32R, kind="ExternalInput").ap()
    d_b1 = nc.dram_tensor("b1r", [128, 4], F32, kind="ExternalInput").ap()
    d_b2 = nc.dram_tensor("b2r", [128, 4], F32, kind="ExternalInput").ap()
    d_id = nc.dram_tensor("ident", [128, 128], F32R, kind="ExternalInput").ap()
    d_ones = nc.dram_tensor("onesr", [1, 128], F32R, kind="ExternalInput").ap()
    # per-feature constants over the merged 64-wide transform axis
    d_cL = nc.dram_tensor("cL", [128, 64], F32, kind="ExternalInput").ap()
    d_cW = nc.dram_tensor("cW", [128, 64], F32, kind="ExternalInput").ap()
    d_xn = nc.dram_tensor("xn", [n_core, 64], F32, kind="ExternalInput").ap()
    d_out = nc.dram_tensor("outputs", [n_core, 128], F32, kind="ExternalOutput").ap()
    d_lad = nc.dram_tensor("lad", [n_core], F32, kind="ExternalOutput").ap()

    with tile.TileContext(nc) as tc:
        with tc.tile_pool(name="const", bufs=1) as cst, \
             tc.tile_pool(name="mlp", bufs=2) as mlp, \
             tc.tile_pool(name="sm", bufs=1) as sm, \
             tc.tile_pool(name="lg", bufs=1) as lg, \
             tc.tile_pool(name="pt", bufs=1, space="PSUM") as ppt, \
             tc.tile_pool(name="pmlp", bufs=3, space="PSUM") as pmlp, \
             tc.tile_pool(name="pl3", bufs=1, space="PSUM") as pl3:

            vec, gps, act = nc.vector, nc.gpsimd, nc.scalar

            # ---- identity feature columns: straight DRAM->DRAM ----
            nc.sync.dma_start(d_out[:, 0:32], d_in[:, 0:32])
            nc.sync.dma_start(d_out[:, 64:96], d_in[:, 64:96])

            # ---- constants ----
            w1t = cst.tile([128, 512], F32R)
            nc.sync.dma_start(w1t[:], d_w1[0:128, :])
            w1b = cst.tile([32, 512], F32R)
            nc.sync.dma_start(w1b[:], d_w1[128:160, :])
            w2t = cst.tile([128, 4, 512], F32R)
            nc.sync.dma_start(w2t[:], d_w2.rearrange("(k p) h -> p k h", p=128))
            w3t = cst.tile([128, 4, 1024], F32R)
            nc.sync.dma_start(w3t[:], d_w3[0:512, :].rearrange("(k p) n -> p k n", p=128))
            w3l = cst.tile([1, 1024], F32R)
            nc.sync.dma_start(w3l[:], d_w3[512:513, :])
            b1t = cst.tile([128, 4], F32)
            nc.sync.dma_start(b1t[:], d_b1[:])
            b2t = cst.tile([128, 4], F32)
            nc.sync.dma_start(b2t[:], d_b2[:])
            idt = cst.tile([128, 128], F32R)
            nc.sync.dma_start(idt[:], d_id[:])
            cL = cst.tile([128, 64], F32)
            nc.sync.dma_start(cL[:], d_cL[:])
            cW = cst.tile([128, 64], F32)
            nc.sync.dma_start(cW[:], d_cW[:])
            ones = cst.tile([1, 128], F32R)
            nc.sync.dma_start(ones[:], d_ones[:])

            for t in range(n_tiles):
                r0 = t * S_TILE

                # ---- inputs ----
                x2 = sm.tile([128, C, 64], F32, tag="x2", bufs=2)
                nc.sync.dma_start(
                    x2[:], d_xn[r0:r0 + S_TILE, :].rearrange("(c p) f -> p c f", p=128))
                nin = sm.tile([128, C, 160], F32R, tag="nin", bufs=2)
                nc.sync.dma_start(
                    nin[:],
                    d_net[r0:r0 + S_TILE, :].rearrange("(c p) f -> p c f", p=128))

                # ---- transpose net_in to feature-major ----
                ntA = sm.tile([128, C, 128], F32R, tag="ntA", bufs=2)
                ntB = sm.tile([32, C, 128], F32R, tag="ntB", bufs=2)
                for c in range(C):
                    pt = ppt.tile([128, 256], F32R, tag="pt")
                    nc.tensor.transpose(pt[:, 0:128], nin[:, c, 0:128],
                                        idt[:])
                    nc.tensor.transpose(pt[0:32, 128:256], nin[:, c, 128:160],
                                        idt[:])
                    act.copy(ntA[:, c, :], pt[:, 0:128])
                    act.copy(ntB[:, c, :], pt[0:32, 128:256])
                ntA_f = ntA[:].rearrange("p c s -> p (c s)")
                ntB_f = ntB[:].rearrange("p c s -> p (c s)")

                # ---- layer 1 ----
                h1t = mlp.tile([128, 4, 512], F32R, tag="h1", bufs=1)
                for mi in range(4):
                    p1 = pmlp.tile([128, 512], F32, tag="pmlp")
                    nc.tensor.matmul(p1[:], w1t[:, mi * 128:(mi + 1) * 128],
                                     ntA_f, start=True, stop=False)
                    nc.tensor.matmul(p1[:], w1b[:, mi * 128:(mi + 1) * 128],
                                     ntB_f, start=False, stop=True)
                    act.activation(h1t[:, mi, :], p1[:], AF.Relu, bias=b1t[:, mi:mi + 1])

                # ---- layer 2 ----
                h2t = mlp.tile([128, 4, 512], F32R, tag="h2", bufs=1)
                for mi in range(4):
                    p2 = pmlp.tile([128, 512], F32, tag="pmlp")
                    for k in range(4):
                        nc.tensor.matmul(p2[:], w2t[:, k, mi * 128:(mi + 1) * 128],
                                         h1t[:, k, :], start=(k == 0), stop=(k == 3))
                    act.activation(h2t[:, mi, :], p2[:], AF.Relu, bias=b2t[:, mi:mi + 1])

                # ---- layer 3 (sample-major params) + nonlinearity extraction ----
                e4 = lg.tile([128, 8, 64, 5], DT_W, tag="e4", bufs=1)
                sp = lg.tile([128, C, 64, 6], DT_W, tag="sp", bufs=1)
                for c in range(C):
                    p3 = pl3.tile([128, 1024], F32, tag="pl3", bufs=2)
                    for k in range(4):
                        lhsT = h2t[:, k, c * 128:(c + 1) * 128]
                        nc.tensor.matmul(p3[:, 0:512], lhsT, w3t[:, k, 0:512],
                                         start=(k == 0), stop=False)
                        nc.tensor.matmul(p3[:, 512:1024], lhsT, w3t[:, k, 512:1024],
                                         start=(k == 0), stop=False)
                    nc.tensor.matmul(p3[:, 0:512], ones[:], w3l[:, 0:512],
                                     start=False, stop=True)
                    nc.tensor.matmul(p3[:, 512:1024], ones[:], w3l[:, 512:1024],
                                     start=False, stop=True)
                    # [p, b, x] view: b=0 -> circular cols 0:512, b=1 -> regular
                    pb = p3[:].rearrange("p (b x) -> p b x", b=2)
                    act.activation(e4[:, c, :, :].rearrange("p (b d) j -> p b d j", b=2),
                                   pb[:, :, 0:160].rearrange("p b (d j) -> p b d j", j=5),
                                   AF.Exp, scale=SCALE)
                    act.activation(e4[:, 4 + c, :, :].rearrange("p (b d) j -> p b d j", b=2),
                                   pb[:, :, 160:320].rearrange("p b (d j) -> p b d j", j=5),
                                   AF.Exp, scale=SCALE)
                    act.activation(sp[:, c, 0:32, 0:5],
                                   pb[:, 0, 320:480].rearrange("p (d j) -> p d j", j=5),
                                   AF.Exp)
                    act.activation(sp[:, c, 32:64, 0:6],
                                   pb[:, 1, 320:512].rearrange("p (d j) -> p d j", j=6),
                                   AF.Exp)

                # softplus finish: sp = Ln(sp + 1), then += MIN_D; circular wrap
                act.activation(sp[:, :, 0:32, 0:5], sp[:, :, 0:32, 0:5], AF.Ln, bias=1.0)
                act.activation(sp[:, :, 32:64, 0:6], sp[:, :, 32:64, 0:6], AF.Ln, bias=1.0)
                vec.tensor_scalar_add(sp[:, :, 0:32, 0:5], sp[:, :, 0:32, 0:5], MIN_D)
                vec.tensor_scalar_add(sp[:, :, 32:64, 0:6], sp[:, :, 32:64, 0:6], MIN_D)
                vec.tensor_copy(sp[:, :, 0:32, 5], sp[:, :, 0:32, 0])

                # ---- normalized widths/heights: wh = MIN_BW + 0.995 * e / E ----
                E2 = sm.tile([128, 8, 64], F32, tag="E2")
                vec.tensor_reduce(E2[:], e4[:], axis=AX.X, op=OP.add)
                rE = sm.tile([128, 8, 64], F32, tag="rE")
                vec.reciprocal_approx_fast(rE[:], E2[:])
                rE16 = sm.tile([128, 8, 64], DT_W, tag="rE16")
                vec.tensor_copy(rE16[:], rE[:])
                wh = lg.tile([128, 8, 64, 5], DT_W, tag="wh", bufs=1)
                vec.scalar_tensor_tensor(
                    wh[:], e4[:], 1.0 - MIN_BW * K,
                    rE16[:].unsqueeze(3).broadcast_to([128, 8, 64, 5]),
                    op0=OP.mult, op1=OP.mult)
                vec.tensor_scalar_add(wh[:], wh[:], MIN_BW)
                w4 = wh[:, 0:4]          # [128, C, 64, 5] widths
                h4 = wh[:, 4:8]          # heights

                # ---- knots, bin masks ----
                Ct = lg.tile([128, C, 64, 4], DT_W, tag="Ct")
                vec.tensor_copy(Ct[:, :, :, 0], w4[:, :, :, 0])
                for j in range(1, 4):
                    vec.tensor_add(Ct[:, :, :, j], Ct[:, :, :, j - 1], w4[:, :, :, j])
                xl = sm.tile([128, C, 64], DT_W, tag="xl")
                act.copy(xl[:], x2[:])
                m_t = lg.tile([128, C, 64, 4], DT_W, tag="m")
                vec.tensor_tensor(m_t[:], xl[:].unsqueeze(3).broadcast_to([128, C, 64, 4]),
                                  Ct[:], op=OP.is_ge)

                # derivative diffs (wrap included via sp col 5)
                dd = lg.tile([128, C, 64, 5], DT_W, tag="dd")
                gps.tensor_tensor(dd[:], sp[:, :, :, 1:6], sp[:, :, :, 0:5],
                                  op=OP.subtract)

                # ---- mask-dot gathers with pairwise adder tree ----
                mf = m_t[:].rearrange("p c d j -> p (c d) j")
                w4f = w4.rearrange("p c d j -> p (c d) j")
                h4f = h4.rearrange("p c d j -> p (c d) j")
                ddf = dd[:].rearrange("p c d j -> p (c d) j")
                VS = [w4f[:, :, 0:4], w4f[:, :, 1:5],
                      h4f[:, :, 0:4], h4f[:, :, 1:5],
                      ddf[:, :, 0:4], ddf[:, :, 1:5]]
                tmp6 = lg.tile([128, 6, CD, 4], DT_W, tag="tmp6")
                for gi, V in enumerate(VS):
                    eng = gps if gi in (2, 3) else vec
                    eng.tensor_tensor(tmp6[:, gi], mf, V, op=OP.mult)
                t6 = tmp6[:].rearrange("p g n (a j) -> p (g n) a j", a=2)
                t2 = lg.tile([128, 6, CD, 2], DT_W, tag="t2")
                vec.tensor_tensor(t2[:].rearrange("p g n j -> p (g n) j"),
                                  t6[:, :, 0, :], t6[:, :, 1, :], op=OP.add)
                g6 = lg.tile([128, 6, CD], F32, tag="g6")
                vec.tensor_tensor(g6[:], t2[:, :, :, 0], t2[:, :, :, 1], op=OP.add)

                # gathered quantities (all [128, CD] f32)
                def smt(name):
                    return sm.tile([128, CD], F32, tag=name, name=name)

                w0f = w4[:, :, :, 0].rearrange("p c d -> p (c d)")
                h0f = h4[:, :, :, 0].rearrange("p c d -> p (c d)")
                xlf = x2[:].rearrange("p c d -> p (c d)")

                s = smt("s")
                vec.tensor_tensor(s[:], xlf, g6[:, 0], op=OP.subtract)
                dwg = smt("dwg")
                gps.tensor_tensor(dwg[:], g6[:, 1], g6[:, 0], op=OP.subtract)
                in_w = smt("in_w")
                vec.tensor_tensor(in_w[:], dwg[:], w0f, op=OP.add)
                dhg = smt("dhg")
                gps.tensor_tensor(dhg[:], g6[:, 3], g6[:, 2], op=OP.subtract)
                in_h = smt("in_h")
                vec.tensor_tensor(in_h[:], dhg[:], h0f, op=OP.add)
                ch0 = g6[:, 2]  # cumh[idx] - bottom
                dk2 = sm.tile([128, CD, 2], F32, tag="dk2")
                spf = sp[:].rearrange("p c d j -> p (c d) j")
                g_pair = g6[:].rearrange("p g n -> p n g")[:, :, 4:6]
                vec.tensor_tensor(dk2[:], g_pair, spf[:, :, 0:2], op=OP.add)
                d_k = dk2[:, :, 0]
                d_k1 = dk2[:, :, 1]

                # ---- rational part (division-free, scaled by in_w^3) ----
                def bin2(name, a, b, op, eng=vec):
                    tt = sm.tile([128, CD], F32, tag=name, name=name)
                    eng.tensor_tensor(tt[:], a, b, op=op)
                    return tt

                u = bin2("u", in_w[:], s[:], OP.subtract, gps)
                su = bin2("su", s[:], u[:], OP.mult)
                s2 = smt("s2")
                act.activation(s2[:], s[:], AF.Square)
                u2 = smt("u2")
                act.activation(u2[:], u[:], AF.Square)
                w2 = smt("w2")
                act.activation(w2[:], in_w[:], AF.Square)
                t1 = bin2("t1", d_k, d_k1, OP.add, gps)
                t1w = bin2("t1w", t1[:], in_w[:], OP.mult, gps)
                hsu = bin2("hsu", in_h[:], su[:], OP.mult)
                hw2 = bin2("hw2", in_h[:], w2[:], OP.mult, gps)
                m1 = bin2("m1", t1w[:], su[:], OP.mult)
                t4 = smt("t4")
                vec.scalar_tensor_tensor(t4[:], hsu[:], -2.0, m1[:],
                                         op0=OP.mult, op1=OP.add)
                D3 = bin2("D3", t4[:], hw2[:], OP.add)
                t5 = bin2("t5", in_h[:], s2[:], OP.mult)
                t5b = bin2("t5b", in_h[:], t5[:], OP.mult)
                aw = bin2("aw", d_k, in_w[:], OP.mult, gps)
                t8 = bin2("t8", hsu[:], aw[:], OP.mult)
                num = bin2("num", t5b[:], t8[:], OP.add)
                rD3 = smt("rD3")
                vec.reciprocal_approx_fast(rD3[:], D3[:])
                y0 = bin2("y0", num[:], rD3[:], OP.mult)
                y1 = bin2("y1", y0[:], ch0, OP.add)
                ysc = sm.tile([128, C, 64], F32, tag="ysc", name="ysc")
                vec.tensor_tensor(ysc[:], y1[:].rearrange("p (c d) -> p c d", c=C),
                                  cW[:].unsqueeze(1).broadcast_to([128, C, 64]),
                                  op=OP.mult)
                yout = sm.tile([128, C, 64], F32, tag="yout", bufs=2)
                vec.tensor_tensor(yout[:], ysc[:],
                                  cL[:].unsqueeze(1).broadcast_to([128, C, 64]),
                                  op=OP.add)

                # logabsdet = 2 ln(in_h) + ln(in_w) + ln(Q) - 2 ln(D3)
                bw = bin2("bw", d_k1, in_w[:], OP.mult, gps)
                q1 = bin2("q1", bw[:], s2[:], OP.mult)
                QQ = smt("QQ")
                vec.scalar_tensor_tensor(QQ[:], hsu[:], 2.0, q1[:],
                                         op0=OP.mult, op1=OP.add)
                q3 = bin2("q3", aw[:], u2[:], OP.mult, gps)
                Q = bin2("Q", QQ[:], q3[:], OP.add)
                lnh = smt("lnh")
                act.activation(lnh[:], in_h[:], AF.Ln)
                lnw = smt("lnw")
                act.activation(lnw[:], in_w[:], AF.Ln)
                lnQ = smt("lnQ")
                act.activation(lnQ[:], Q[:], AF.Ln)
                lnD = smt("lnD")
                act.activation(lnD[:], D3[:], AF.Ln)
                l1 = smt("l1")
                vec.scalar_tensor_tensor(l1[:], lnh[:], 2.0, lnw[:],
                                         op0=OP.mult, op1=OP.add)
                lad = smt("lad")
                vec.scalar_tensor_tensor(lad[:], lnD[:], -2.0, lnQ[:],
                                         op0=OP.mult, op1=OP.add)
                ladf = bin2("ladf", l1[:], lad[:], OP.add)

                # ---- outputs ----
                dout_t = d_out[r0:r0 + S_TILE, :].rearrange("(c p) f -> p c f", p=128)
                nc.sync.dma_start(dout_t[:, :, 32:64], yout[:, :, 0:32])
                nc.sync.dma_start(dout_t[:, :, 96:128], yout[:, :, 32:64])
                lsum = sm.tile([128, C], F32, tag="lsum", bufs=2)
                vec.tensor_reduce(lsum[:], ladf[:].rearrange("p (c d) -> p c d", c=C),
                                  axis=AX.X, op=OP.add)
                nc.sync.dma_start(
                    d_lad[r0:r0 + S_TILE].rearrange("(c p) -> p c", p=128), lsum[:])

    nc.compile()
    return nc


def prep_weights(W1, b1, W2, b2, W3, b3):
    """Host-side: permute+pad W3 columns, append b3 row, build const tiles."""
    perm = np.zeros(1024, dtype=np.int64)
    valid = np.zeros(1024, dtype=bool)
    for d in range(32):
        for j in range(5):
            perm[0 + d * 5 + j] = d * 15 + j          # uw_c
            perm[160 + d * 5 + j] = d * 15 + 5 + j    # uh_c
            perm[320 + d * 5 + j] = d * 15 + 10 + j   # ud_c
            perm[512 + d * 5 + j] = 480 + d * 16 + j        # uw_r
            perm[672 + d * 5 + j] = 480 + d * 16 + 5 + j    # uh_r
        for j in range(6):
            perm[832 + d * 6 + j] = 480 + d * 16 + 10 + j   # ud_r
    valid[0:480] = True
    valid[512:1024] = True

    W3p = np.zeros((512, 1024), np.float32)
    b3p = np.zeros((1024,), np.float32)
    W3p[:, valid] = W3[:, perm[valid]]
    b3p[valid] = b3[perm[valid]]
    W3b = np.concatenate([W3p, b3p[None, :]], axis=0).astype(np.float32)

    # per-feature constants on the merged 64-wide axis (first 32 circular)
    width = np.where(np.arange(64) < 32, 2.0 * PI, 2.0 * TAIL).astype(np.float32)
    cL = np.broadcast_to(-width / 2.0, (128, 64)).copy()
    cW = np.broadcast_to(width, (128, 64)).copy()

    return {
        "W1f": np.ascontiguousarray(W1, dtype=np.float32),
        "W2f": np.ascontiguousarray(W2, dtype=np.float32),
        "W3b": W3b,
        "b1r": b1.reshape(4, 128).T.copy().astype(np.float32),
        "b2r": b2.reshape(4, 128).T.copy().astype(np.float32),
        "ident": np.eye(128, dtype=np.float32),
        "onesr": np.ones((1, 128), dtype=np.float32),
        "cL": cL, "cW": cW,
    }


def make_xn(inputs):
    xc = (inputs[:, 32:64] + PI) / (2.0 * PI)
    xr = (inputs[:, 96:128] + TAIL) / (2.0 * TAIL)
    return np.ascontiguousarray(np.concatenate([xc, xr], axis=1).astype(np.float32))


def make_net32(inputs, context):
    ic = inputs[:, 0:32]
    return np.ascontiguousarray(np.concatenate(
        [np.cos(ic), np.sin(ic), inputs[:, 64:96], context],
        axis=1).astype(np.float32))


_NC_CACHE = {}


def _get_nc(n_core):
    if n_core not in _NC_CACHE:
        _NC_CACHE[n_core] = build_nc(n_core)
    return _NC_CACHE[n_core]


def kernel(inputs, context, W1, b1, W2, b2, W3, b3):
    inputs = np.ascontiguousarray(np.asarray(inputs, dtype=np.float32))
    context = np.asarray(context, dtype=np.float32)
    wmap = prep_weights(np.asarray(W1, np.float32), np.asarray(b1, np.float32),
                        np.asarray(W2, np.float32), np.asarray(b2, np.float32),
                        np.asarray(W3, np.float32), np.asarray(b3, np.float32))
    net32 = make_net32(inputs, context)
    xn = make_xn(inputs)

    nc = _get_nc(N_CORE)
    in_maps = []
    for c in range(N_CORES):
        sl = slice(c * N_CORE, (c + 1) * N_CORE)
        m = dict(wmap)
        m["inputs"] = inputs[sl]
        m["net32"] = net32[sl]
        m["xn"] = xn[sl]
        in_maps.append(m)

    res = run_bass_kernel_spmd(nc, in_maps, list(range(N_CORES)))
    outputs = np.concatenate([res.results[c]["outputs"] for c in range(N_CORES)], 0)
    lad = np.concatenate([res.results[c]["lad"] for c in range(N_CORES)], 0)
    return outputs, lad
